# revision 8
# baseline (speedup 1.0000x reference)
"""Self-attention kernel for TRN2, data-parallel over batch (8 cores), fp8.

Per core (one batch element x[2048, 512]):
  - x loaded fp32 (residual), cast bf16 on ScalarE, transposed to xT by
    the DMA XBAR (dma_start_transpose) - no TensorE transpose-mode work.
  - q/k/v projections run bf16 (xT/weights bf16); their PSUM epilogues
    emit fp8 qT/kT/v for the attention core.
  - scores and PV run fp8 with perf_mode=DoubleRow (contraction pairs of
    128-chunks -> ~2x TensorE); scores are computed TRANSPOSED (sT[s,t])
    so the exp output feeds PV directly; exp = e^{score/16 - 2} (bias
    cancels in normalization), fused over two PSUM banks per activation.
  - row sums come free via a ones-column in v; the reciprocal is folded
    into the bf16 cast of a (per-partition scalar); a is transposed by
    the DMA XBAR and the output projection runs bf16.
  - qT for block tb+1, the output projection of block tb-1, and the
    residual-bias adds are interleaved into block tb's score streak.
  - biases: bq/bk exact via per-partition add; bv/ba folded on the HOST
    into bc = Wa^T bv + ba, added into the fp32 residual x (exact:
    attention rows sum to 1).

PSUM accumulation fp32 everywhere; softmax/normalize/residual fp32.
"""

import ml_dtypes
import numpy as np

import concourse.bass as bass
import concourse.mybir as mybir
import concourse.tile as tile
from concourse import bacc
from concourse.bass import ds, ts
from concourse.bass_utils import run_bass_kernel_spmd
from concourse.masks import make_identity

F32 = mybir.dt.float32
BF16 = mybir.dt.bfloat16
F8 = mybir.dt.float8e4
AF = mybir.ActivationFunctionType
DR = mybir.MatmulPerfMode.DoubleRow

B, T, C, U, P = 8, 2048, 512, 256, 128
TC = T // P    # 16 row tiles
CCH = C // P   # 4 c-chunks
UCH = U // P   # 2 u-chunks
TBLK = 512     # t-block for attention
NTB = T // TBLK  # 4
VF = U + 16    # v free dim padded so the pair-dim stride is 16B-aligned
SCALE = 1.0 / float(np.sqrt(U))
EXPB = -2.0    # exp bias; cancels in row-sum normalization

_cache = {}


def _build_kernel(tc):
    nc = tc.nc
    x = nc.dram_tensor("x", [T, C], F32, kind="ExternalInput").ap()
    Wqb = nc.dram_tensor("Wqb", [P, CCH, U], BF16, kind="ExternalInput").ap()
    Wkb = nc.dram_tensor("Wkb", [P, CCH, U], BF16, kind="ExternalInput").ap()
    Wvb = nc.dram_tensor("Wvb", [P, CCH, U], BF16, kind="ExternalInput").ap()
    Wab = nc.dram_tensor("Wab", [P, UCH, C], BF16, kind="ExternalInput").ap()
    bq = nc.dram_tensor("bq", [U], F32, kind="ExternalInput").ap()
    bk = nc.dram_tensor("bk", [U], F32, kind="ExternalInput").ap()
    bcrep = nc.dram_tensor("bcrep", [P, C], F32, kind="ExternalInput").ap()
    out = nc.dram_tensor("out", [T, C], F32, kind="ExternalOutput").ap()

    consts = tc.alloc_tile_pool(name="consts", bufs=1)
    persist = tc.alloc_tile_pool(name="persist", bufs=1)

    identity = consts.tile([P, P], BF16)
    make_identity(nc, identity)

    # warm the ACT exp table early (one-time ~2.7us table load)
    dex = consts.tile([P, 1], F32)
    nc.vector.memset(dex, 0.0)
    expb = consts.tile([P, 1], F32)
    nc.vector.memset(expb, EXPB)
    dex2 = consts.tile([P, 1], F32)
    nc.scalar.activation(out=dex2, in_=dex, func=AF.Exp, bias=dex[:, 0:1],
                         scale=1.0)

    # persistent tensors
    x_sb = persist.tile([P, TC, C], F32)      # x rows (+bc), fp32 residual
    xT_bf = persist.tile([P, CCH, T], BF16)   # x^T  (c on partitions)
    qT_f8 = persist.tile([P, UCH, T], F8)     # q^T  (u on partitions)
    kT_f8 = persist.tile([P, UCH, T], F8)     # k^T
    v_sb = persist.tile([P, TC, VF], F8)      # v rows + ones col + pad
    aT_bf = persist.tile([P, UCH, T], BF16)   # a^T (normalized)
    nc.vector.memset(v_sb[:, :, U:VF], 0.0)
    nc.vector.memset(v_sb[:, :, U:U + 1], 1.0)

    # input DMAs: ALL on the sync queue, in consumption order
    xr = x.rearrange("(tt p) c -> p tt c", p=P)
    for k in range(TC // 2):
        nc.sync.dma_start(out=x_sb[:, 2 * k:2 * k + 2, :],
                          in_=xr[:, 2 * k:2 * k + 2, :])
    Wq_bf = consts.tile([P, CCH, U], BF16)
    nc.sync.dma_start(out=Wq_bf, in_=Wqb)
    Wk_bf = consts.tile([P, CCH, U], BF16)
    nc.sync.dma_start(out=Wk_bf, in_=Wkb)
    Wv_bf = consts.tile([P, CCH, U], BF16)
    nc.sync.dma_start(out=Wv_bf, in_=Wvb)
    Wa_bf = consts.tile([P, UCH, C], BF16)
    nc.sync.dma_start(out=Wa_bf, in_=Wab)
    bq_sb = consts.tile([P, UCH], F32)
    nc.sync.dma_start(out=bq_sb, in_=bq.rearrange("(uc p) -> p uc", p=P))
    bk_sb = consts.tile([P, UCH], F32)
    nc.sync.dma_start(out=bk_sb, in_=bk.rearrange("(uc p) -> p uc", p=P))
    bc_sb = consts.tile([P, C], F32)
    nc.sync.dma_start(out=bc_sb, in_=bcrep)

    # PE warmup during the x DMA
    with tc.tile_pool(name="warm", bufs=1, space="PSUM") as warm_pool:
        wtile = warm_pool.tile([P, P], F32, name="warmup")
        for i in range(12):
            nc.tensor.matmul(wtile, lhsT=identity, rhs=identity,
                             start=(i == 0), stop=(i == 11))

    # --- phases 1+2 interleaved per t-block: cast + XBAR transpose for
    # the block's four x tiles, then its kT (+ qT block 0) and v ---
    def proj_group(wpool, W_bf, bias_sb, dst, uc, tb, eng_act):
        wps = wpool.tile([P, TBLK], F32, tag="wps", name="wps")
        for cc in range(CCH):
            nc.tensor.matmul(
                wps,
                lhsT=W_bf[:, cc, ts(uc, P)],
                rhs=xT_bf[:, cc, ds(tb * TBLK, TBLK)],
                start=(cc == 0), stop=(cc == CCH - 1),
            )
        if eng_act:
            nc.scalar.activation(
                out=dst[:, uc, ds(tb * TBLK, TBLK)], in_=wps,
                func=AF.Identity, bias=bias_sb[:, uc:uc + 1], scale=1.0,
            )
        else:
            nc.vector.tensor_scalar(
                out=dst[:, uc, ds(tb * TBLK, TBLK)], in0=wps,
                scalar1=bias_sb[:, uc:uc + 1], scalar2=None,
                op0=mybir.AluOpType.add,
            )

    with tc.tile_pool(name="xbf", bufs=3) as xbf_pool, \
         tc.tile_pool(name="wpsum", bufs=3, space="PSUM") as wpsum, \
         tc.tile_pool(name="vpsum", bufs=2, space="PSUM") as vpsum:
        for tb in range(NTB):
            for tt in range(tb * 4, tb * 4 + 4):
                x_bf = xbf_pool.tile([P, C], BF16, tag="xbf")
                nc.scalar.copy(out=x_bf, in_=x_sb[:, tt, :])
                nc.sync.dma_start_transpose(
                    out=xT_bf[:, :, ts(tt, P)], in_=x_bf
                )
            for uc in range(UCH):
                proj_group(wpsum, Wk_bf, bk_sb, kT_f8, uc, tb, uc == 0)
            if tb == 0:
                for uc in range(UCH):
                    proj_group(wpsum, Wq_bf, bq_sb, qT_f8, uc, 0, uc == 0)
            # v for tiles of this block: two row tiles share one PSUM
            # bank (the second opens with start=False so the bank-wide
            # clear of the first start doesn't zero it)
            for h in range(2):
                t0 = tb * 4 + 2 * h
                vps = vpsum.tile([P, 2, U], F32, tag="vps")
                for j in range(2):
                    for cc in range(CCH):
                        nc.tensor.matmul(
                            vps[:, j, :],
                            lhsT=xT_bf[:, cc, ts(t0 + j, P)],
                            rhs=Wv_bf[:, cc, :],
                            start=(j == 0 and cc == 0),
                            stop=(j == 1 and cc == CCH - 1),
                        )
                if h == 0:
                    nc.scalar.copy(out=v_sb[:, t0:t0 + 2, 0:U], in_=vps)
                else:
                    nc.vector.tensor_copy(
                        out=v_sb[:, t0:t0 + 2, 0:U], in_=vps
                    )

    # --- phase 3: attention ---
    spsum = tc.alloc_tile_pool(name="spsum", bufs=2, space="PSUM")
    apsum = tc.alloc_tile_pool(name="apsum", bufs=2, space="PSUM")
    p_pool = tc.alloc_tile_pool(name="p_pool", bufs=10)
    abf_pool = tc.alloc_tile_pool(name="abf_pool", bufs=6)
    rcp_pool = tc.alloc_tile_pool(name="rcp_pool", bufs=3)
    y_pool = tc.alloc_tile_pool(name="y_pool", bufs=2)

    def norm_cast(apss, abfs, tb, tsl):
        """rcp of row sum; a_bf = aps * rcp (normalized) fp32->bf16; then
        XBAR-transpose a_bf into aT."""
        aps = apss[tsl]
        rcp = rcp_pool.tile([P, 1], F32, tag="rcp")
        nc.vector.reciprocal(rcp, aps[:, U:U + 1])
        a_bf = abf_pool.tile([P, U], BF16, tag="abf")
        nc.vector.tensor_scalar(
            out=a_bf, in0=aps[:, 0:U], scalar1=rcp, scalar2=None,
            op0=mybir.AluOpType.mult,
        )
        abfs[tsl] = a_bf
        row0 = tb * TBLK + tsl * P
        nc.sync.dma_start_transpose(out=aT_bf[:, :, ds(row0, P)], in_=a_bf)

    def deferred_work(tb):
        """Output projection + residual (tb); one DMA per two row tiles."""
        chunks = []
        y2box = [None]
        for tsl in range(NTB):
            def fchunk(tsl=tsl, tb=tb, y2box=y2box):
                row0 = tb * TBLK + tsl * P
                yps = apsum.tile([P, TBLK], F32, tag="misc", name="yps")
                for uc in range(UCH):
                    nc.tensor.matmul(
                        yps, lhsT=aT_bf[:, uc, ds(row0, P)],
                        rhs=Wa_bf[:, uc, :],
                        start=(uc == 0), stop=(uc == UCH - 1),
                    )
                if tsl % 2 == 0:
                    y2box[0] = y_pool.tile([P, 2, C], F32, tag="ysb",
                                           name="y2")
                y2 = y2box[0]
                nc.vector.tensor_add(
                    out=y2[:, tsl % 2, :], in0=yps,
                    in1=x_sb[:, tb * NTB + tsl, :]
                )
                if tsl % 2 == 1:
                    orow = tb * TBLK + (tsl - 1) * P
                    nc.sync.dma_start(
                        out=out[ds(orow, 2 * P), :].rearrange(
                            "(j p) c -> p j c", p=P),
                        in_=y2,
                    )
            chunks.append(fchunk)
        return chunks

    deferred = []
    for tb in range(NTB):
        pts = []
        abfs = [None] * NTB
        apss = [None] * NTB
        for tsl in (0, 1):
            apss[tsl] = apsum.tile([P, VF], F32, tag="acc", name="apsA")
        todo = list(deferred)  # deferred chunks from tb-1
        for scp in range(8):
            sps = spsum.tile([P, 2, TBLK], F32, tag="sps", name="sps")
            for j in range(2):
                nc.tensor.matmul(
                    sps[:, j, :],
                    lhsT=kT_f8[:, :, ts(2 * scp + j, P)],
                    rhs=qT_f8[:, :, ds(tb * TBLK, TBLK)],
                    start=True, stop=True, perf_mode=DR,
                )
            pt = p_pool.tile([P, 2, TBLK], F8, tag="pt")
            nc.scalar.activation(out=pt, in_=sps, func=AF.Exp,
                                 bias=expb[:, 0:1], scale=SCALE)
            pts.append(pt)
            # PV sweep A (row tiles 0,1), one pair behind the exp
            if scp >= 1:
                for tsl in (0, 1):
                    nc.tensor.matmul(
                        apss[tsl],
                        lhsT=pts[scp - 1][:, :, ts(tsl, P)],
                        rhs=v_sb[:, 2 * (scp - 1):2 * scp, :],
                        start=(scp == 1), stop=False, perf_mode=DR,
                    )
            # interleave deferred output-projection chunks of tb-1
            if scp >= 3 and todo and len(todo) > (7 - scp):
                todo.pop(0)()
            # residual bias add for this block's x tiles (needed by the
            # output projection one block later)
            if scp >= 4:
                tt = tb * 4 + scp - 4
                nc.vector.tensor_add(out=x_sb[:, tt, :],
                                     in0=x_sb[:, tt, :], in1=bc_sb)
        for tsl in (0, 1):
            nc.tensor.matmul(
                apss[tsl], lhsT=pts[7][:, :, ts(tsl, P)],
                rhs=v_sb[:, 14:16, :], start=False, stop=True, perf_mode=DR,
            )
        while todo:
            todo.pop(0)()
        norm_cast(apss, abfs, tb, 0)
        norm_cast(apss, abfs, tb, 1)
        # PV sweep B (row tiles 2,3) over the retained p tiles
        for tsl in (2, 3):
            apss[tsl] = apsum.tile([P, VF], F32, tag="acc", name="apsB")
        for scp in range(8):
            for tsl in (2, 3):
                nc.tensor.matmul(
                    apss[tsl],
                    lhsT=pts[scp][:, :, ts(tsl, P)],
                    rhs=v_sb[:, 2 * scp:2 * scp + 2, :],
                    start=(scp == 0), stop=(scp == 7), perf_mode=DR,
                )
        norm_cast(apss, abfs, tb, 2)
        norm_cast(apss, abfs, tb, 3)
        # produce qT for the NEXT block inside this streak
        if tb + 1 < NTB:
            for uc in range(UCH):
                wps = apsum.tile([P, TBLK], F32, tag="misc", name="qps")
                for cc in range(CCH):
                    nc.tensor.matmul(
                        wps,
                        lhsT=Wq_bf[:, cc, ts(uc, P)],
                        rhs=xT_bf[:, cc, ds((tb + 1) * TBLK, TBLK)],
                        start=(cc == 0), stop=(cc == CCH - 1),
                    )
                nc.scalar.activation(
                    out=qT_f8[:, uc, ds((tb + 1) * TBLK, TBLK)],
                    in_=wps,
                    func=AF.Identity, bias=bq_sb[:, uc:uc + 1], scale=1.0,
                )
        if tb < NTB - 1:
            deferred = deferred_work(tb)
        else:
            # last block: emit immediately to shorten the tail
            for chunk in deferred_work(tb):
                chunk()

    for pool in (y_pool, rcp_pool, abf_pool, p_pool,
                 apsum, spsum, persist, consts):
        pool.release()


def _get_nc():
    if "nc" not in _cache:
        nc = bacc.Bacc("TRN2", target_bir_lowering=False, debug=False)
        with tile.TileContext(nc) as tc:
            _build_kernel(tc)
        nc.compile()
        _cache["nc"] = nc
    return _cache["nc"]


def _wb(w, chunks):
    """fp32 [K, N] -> bf16 [P, K//P, N] with K-chunk layout for lhsT."""
    wb = w.reshape(chunks, P, -1).transpose(1, 0, 2)
    return np.ascontiguousarray(wb.astype(ml_dtypes.bfloat16))


def _host_inputs(inputs):
    f32 = np.float32
    Wa = np.ascontiguousarray(np.asarray(inputs["Wa"], dtype=f32))
    bc = np.asarray(inputs["bv"], dtype=f32) @ Wa + np.asarray(
        inputs["ba"], dtype=f32
    )
    bcrep = np.ascontiguousarray(
        np.broadcast_to(bc[None, :], (P, C)), dtype=f32
    )
    shared = {
        "Wqb": _wb(np.asarray(inputs["Wq"], dtype=f32), CCH),
        "Wkb": _wb(np.asarray(inputs["Wk"], dtype=f32), CCH),
        "Wvb": _wb(np.asarray(inputs["Wv"], dtype=f32), CCH),
        "Wab": _wb(Wa, UCH),
        "bq": np.ascontiguousarray(np.asarray(inputs["bq"], dtype=f32)),
        "bk": np.ascontiguousarray(np.asarray(inputs["bk"], dtype=f32)),
        "bcrep": bcrep,
    }
    xs = np.ascontiguousarray(np.asarray(inputs["x"], dtype=f32))
    return [dict(shared, x=xs[b]) for b in range(B)]


def kernel(**inputs):
    nc = _get_nc()
    in_maps = _host_inputs(inputs)
    res = run_bass_kernel_spmd(nc, in_maps, core_ids=list(range(B)))
    return np.stack([res.results[b]["out"] for b in range(B)], axis=0)


# revision 9
# speedup vs baseline: 1.2528x; 1.2528x over previous
"""Self-attention kernel for TRN2, data-parallel over batch (8 cores), fp8.

Per core (one batch element x[2048, 512]):
  - x loaded fp32 (residual), cast bf16 on ScalarE, transposed on TensorE
    to xT (c on partitions), stored fp8.
  - q/k/v projections and all attention matmuls run fp8 with
    perf_mode=DoubleRow (contraction pairs of 128-chunks -> ~2x TensorE).
  - scores computed TRANSPOSED (sT[s,t]) so the exp output feeds PV
    directly; exp = e^{score/16 - 2} (bias cancels in normalization),
    fused over two PSUM banks per activation.
  - PV runs all four 128-row output accumulators lag-1 behind the exp so
    there is no drain tail; row sums come free via a ones-column in v;
    the reciprocal is folded into the bf16 cast of a (per-partition
    scalar), so the output projection needs no further scaling.
  - qT for block tb+1, transposes/output-projection of block tb-1, and
    the residual-bias adds are interleaved into block tb's score streak.
  - biases: bq/bk exact via per-partition add; bv/ba folded on the HOST
    into bc = Wa^T bv + ba, added into the fp32 residual x (exact:
    attention rows sum to 1).

Matmul inputs fp8e4, PSUM accumulation fp32, softmax/normalize/residual fp32.
"""

import ml_dtypes
import numpy as np

import concourse.bass as bass
import concourse.mybir as mybir
import concourse.tile as tile
from concourse import bacc
from concourse.bass import ds, ts
from concourse.bass_utils import run_bass_kernel_spmd
from concourse.masks import make_identity

F32 = mybir.dt.float32
BF16 = mybir.dt.bfloat16
F8 = mybir.dt.float8e4
AF = mybir.ActivationFunctionType
DR = mybir.MatmulPerfMode.DoubleRow

B, T, C, U, P = 8, 2048, 512, 256, 128
TC = T // P    # 16 row tiles
CCH = C // P   # 4 c-chunks
UCH = U // P   # 2 u-chunks
TBLK = 512     # t-block for attention
NTB = T // TBLK  # 4
VF = U + 16    # v free dim padded so the pair-dim stride is 16B-aligned
SCALE = 1.0 / float(np.sqrt(U))
EXPB = -2.0    # exp bias; cancels in row-sum normalization

_cache = {}


def _build_kernel(tc):
    nc = tc.nc
    x = nc.dram_tensor("x", [T, C], F32, kind="ExternalInput").ap()
    Wq8 = nc.dram_tensor("Wq8", [P, CCH, U], F8, kind="ExternalInput").ap()
    Wk8 = nc.dram_tensor("Wk8", [P, CCH, U], F8, kind="ExternalInput").ap()
    Wv8 = nc.dram_tensor("Wv8", [P, CCH, U], F8, kind="ExternalInput").ap()
    Wa8 = nc.dram_tensor("Wa8", [P, UCH, C], F8, kind="ExternalInput").ap()
    bq = nc.dram_tensor("bq", [U], F32, kind="ExternalInput").ap()
    bk = nc.dram_tensor("bk", [U], F32, kind="ExternalInput").ap()
    bcrep = nc.dram_tensor("bcrep", [P, C], F32, kind="ExternalInput").ap()
    out = nc.dram_tensor("out", [T, C], F32, kind="ExternalOutput").ap()

    consts = tc.alloc_tile_pool(name="consts", bufs=1)
    persist = tc.alloc_tile_pool(name="persist", bufs=1)

    identity = consts.tile([P, P], BF16)
    make_identity(nc, identity)

    # warm the ACT exp table early (one-time ~2.7us table load)
    dex = consts.tile([P, 1], F32)
    nc.vector.memset(dex, 0.0)
    expb = consts.tile([P, 1], F32)
    nc.vector.memset(expb, EXPB)
    dex2 = consts.tile([P, 1], F32)
    nc.scalar.activation(out=dex2, in_=dex, func=AF.Exp, bias=dex[:, 0:1],
                         scale=1.0)

    # persistent tensors
    x_sb = persist.tile([P, TC, C], F32)      # x rows (+bc), fp32 residual
    xT_f8 = persist.tile([P, CCH, T], F8)     # x^T  (c on partitions)
    qT_f8 = persist.tile([P, UCH, T], F8)     # q^T  (u on partitions)
    kT_f8 = persist.tile([P, UCH, T], F8)     # k^T
    v_sb = persist.tile([P, TC, VF], F8)      # v rows + ones col + pad
    aT_f8 = persist.tile([P, UCH, T], F8)     # a^T (normalized)
    nc.vector.memset(v_sb[:, :, U:VF], 0.0)
    nc.vector.memset(v_sb[:, :, U:U + 1], 1.0)

    # input DMAs: ALL on the sync queue, in consumption order
    xr = x.rearrange("(tt p) c -> p tt c", p=P)
    for k in range(TC // 2):
        nc.sync.dma_start(out=x_sb[:, 2 * k:2 * k + 2, :],
                          in_=xr[:, 2 * k:2 * k + 2, :])
    Wq_f8 = consts.tile([P, CCH, U], F8)
    nc.sync.dma_start(out=Wq_f8, in_=Wq8)
    Wk_f8 = consts.tile([P, CCH, U], F8)
    nc.sync.dma_start(out=Wk_f8, in_=Wk8)
    Wv_f8 = consts.tile([P, CCH, U], F8)
    nc.sync.dma_start(out=Wv_f8, in_=Wv8)
    Wa_f8 = consts.tile([P, UCH, C], F8)
    nc.sync.dma_start(out=Wa_f8, in_=Wa8)
    bq_sb = consts.tile([P, UCH], F32)
    nc.sync.dma_start(out=bq_sb, in_=bq.rearrange("(uc p) -> p uc", p=P))
    bk_sb = consts.tile([P, UCH], F32)
    nc.sync.dma_start(out=bk_sb, in_=bk.rearrange("(uc p) -> p uc", p=P))
    bc_sb = consts.tile([P, C], F32)
    nc.sync.dma_start(out=bc_sb, in_=bcrep)

    # PE warmup during the x DMA
    with tc.tile_pool(name="warm", bufs=1, space="PSUM") as warm_pool:
        wtile = warm_pool.tile([P, P], F32, name="warmup")
        for i in range(12):
            nc.tensor.matmul(wtile, lhsT=identity, rhs=identity,
                             start=(i == 0), stop=(i == 11))

    # --- phases 1+2 interleaved per t-block: transposes for a block's
    # four x tiles, then its kT (+ qT for block 0) and v projections, so
    # projection matmuls overlap the x DMA stream ---
    def proj_group(wpool, W_f8, bias_sb, dst, uc, tb, eng_act):
        wps = wpool.tile([P, TBLK], F32, tag="wps", name="wps")
        for i in range(2):
            nc.tensor.matmul(
                wps,
                lhsT=W_f8[:, 2 * i:2 * i + 2, ts(uc, P)],
                rhs=xT_f8[:, 2 * i:2 * i + 2, ds(tb * TBLK, TBLK)],
                start=(i == 0), stop=(i == 1), perf_mode=DR,
            )
        if eng_act:
            nc.scalar.activation(
                out=dst[:, uc, ds(tb * TBLK, TBLK)], in_=wps,
                func=AF.Identity, bias=bias_sb[:, uc:uc + 1], scale=1.0,
            )
        else:
            nc.vector.tensor_scalar(
                out=dst[:, uc, ds(tb * TBLK, TBLK)], in0=wps,
                scalar1=bias_sb[:, uc:uc + 1], scalar2=None,
                op0=mybir.AluOpType.add,
            )

    with tc.tile_pool(name="xbf", bufs=3) as xbf_pool, \
         tc.tile_pool(name="tpsum", bufs=3, space="PSUM") as tpsum, \
         tc.tile_pool(name="wpsum", bufs=3, space="PSUM") as wpsum, \
         tc.tile_pool(name="vpsum", bufs=2, space="PSUM") as vpsum:
        for tb in range(NTB):
            for tt in range(tb * 4, tb * 4 + 4):
                x_bf = xbf_pool.tile([P, C], BF16, tag="xbf")
                nc.scalar.copy(out=x_bf, in_=x_sb[:, tt, :])
                # transpose via NORMAL matmul against identity (exact for
                # bf16, pipelines ~2.5x faster than transpose-mode and
                # keeps the HAM activity monitor warm)
                tps = tpsum.tile([P, CCH, P], F32, tag="tps")
                for cc in range(CCH):
                    nc.tensor.matmul(
                        tps[:, cc, :], lhsT=x_bf[:, ts(cc, P)], rhs=identity,
                        start=(cc == 0), stop=(cc == CCH - 1),
                    )
                nc.vector.tensor_copy(out=xT_f8[:, :, ts(tt, P)], in_=tps)
            for uc in range(UCH):
                proj_group(wpsum, Wk_f8, bk_sb, kT_f8, uc, tb, uc == 0)
            if tb == 0:
                for uc in range(UCH):
                    proj_group(wpsum, Wq_f8, bq_sb, qT_f8, uc, 0, uc == 0)
            # v for tiles of this block: two row tiles share one PSUM
            # bank (the second pair opens with start=False so the
            # bank-wide clear of the first start doesn't zero it)
            for h in range(2):
                t0 = tb * 4 + 2 * h
                vps = vpsum.tile([P, 2, U], F32, tag="vps")
                for j in range(2):
                    for i in range(2):
                        nc.tensor.matmul(
                            vps[:, j, :],
                            lhsT=xT_f8[:, 2 * i:2 * i + 2, ts(t0 + j, P)],
                            rhs=Wv_f8[:, 2 * i:2 * i + 2, :],
                            start=(j == 0 and i == 0),
                            stop=(j == 1 and i == 1), perf_mode=DR,
                        )
                if h == 0:
                    nc.scalar.copy(out=v_sb[:, t0:t0 + 2, 0:U], in_=vps)
                else:
                    nc.vector.tensor_copy(
                        out=v_sb[:, t0:t0 + 2, 0:U], in_=vps
                    )

    # --- phase 3: attention ---
    # PSUM: spsum 2x(2 banks) for scores, apsum "acc" 2x(1 bank) holding
    # two 128-row PV accumulators per bank (second half opens with
    # start=False so the bank clear of the first doesn't zero it), and
    # "misc" 2x(1 bank) for a-transposes / output projection / next-qT.
    spsum = tc.alloc_tile_pool(name="spsum", bufs=2, space="PSUM")
    apsum = tc.alloc_tile_pool(name="apsum", bufs=2, space="PSUM")
    p_pool = tc.alloc_tile_pool(name="p_pool", bufs=10)
    abf_pool = tc.alloc_tile_pool(name="abf_pool", bufs=6)
    rcp_pool = tc.alloc_tile_pool(name="rcp_pool", bufs=3)
    y_pool = tc.alloc_tile_pool(name="y_pool", bufs=2)

    def norm_cast(apss, abfs, tsl):
        """rcp of row sum, then a_bf = aps * rcp (normalized), fp32->bf16."""
        aps = apss[tsl]
        rcp = rcp_pool.tile([P, 1], F32, tag="rcp")
        nc.vector.reciprocal(rcp, aps[:, U:U + 1])
        a_bf = abf_pool.tile([P, U], BF16, tag="abf")
        nc.vector.tensor_scalar(
            out=a_bf, in0=aps[:, 0:U], scalar1=rcp, scalar2=None,
            op0=mybir.AluOpType.mult,
        )
        abfs[tsl] = a_bf

    def deferred_work(tb, abfs):
        """Transposes of a (tb), then output projection + residual (tb).
        y tiles pair up for one DMA per two row tiles."""
        chunks = []
        y2box = [None]
        for tsl in range(NTB):
            def tchunk(tsl=tsl, tb=tb, abfs=abfs):
                row0 = tb * TBLK + tsl * P
                atps = apsum.tile([P, UCH, P], F32, tag="misc", name="atps")
                for uc in range(UCH):
                    nc.tensor.matmul(
                        atps[:, uc, :], lhsT=abfs[tsl][:, ts(uc, P)],
                        rhs=identity,
                        start=(uc == 0), stop=(uc == UCH - 1),
                    )
                nc.vector.tensor_copy(out=aT_f8[:, :, ds(row0, P)], in_=atps)
            chunks.append(tchunk)
        for tsl in range(NTB):
            def fchunk(tsl=tsl, tb=tb, y2box=y2box):
                row0 = tb * TBLK + tsl * P
                yps = apsum.tile([P, TBLK], F32, tag="misc", name="yps")
                nc.tensor.matmul(
                    yps, lhsT=aT_f8[:, :, ds(row0, P)], rhs=Wa_f8[:, :, :],
                    start=True, stop=True, perf_mode=DR,
                )
                if tsl % 2 == 0:
                    y2box[0] = y_pool.tile([P, 2, C], F32, tag="ysb",
                                           name="y2")
                y2 = y2box[0]
                nc.vector.tensor_add(
                    out=y2[:, tsl % 2, :], in0=yps,
                    in1=x_sb[:, tb * NTB + tsl, :]
                )
                if tsl % 2 == 1:
                    orow = tb * TBLK + (tsl - 1) * P
                    nc.sync.dma_start(
                        out=out[ds(orow, 2 * P), :].rearrange(
                            "(j p) c -> p j c", p=P),
                        in_=y2,
                    )
            chunks.append(fchunk)
        return chunks

    deferred = []
    for tb in range(NTB):
        pts = []
        abfs = [None] * NTB
        apss = [None] * NTB
        for tsl in (0, 1):
            apss[tsl] = apsum.tile([P, VF], F32, tag="acc", name="apsA")
        todo = list(deferred)  # deferred chunks from tb-1
        for scp in range(8):
            sps = spsum.tile([P, 2, TBLK], F32, tag="sps", name="sps")
            for j in range(2):
                nc.tensor.matmul(
                    sps[:, j, :],
                    lhsT=kT_f8[:, :, ts(2 * scp + j, P)],
                    rhs=qT_f8[:, :, ds(tb * TBLK, TBLK)],
                    start=True, stop=True, perf_mode=DR,
                )
            pt = p_pool.tile([P, 2, TBLK], F8, tag="pt")
            nc.scalar.activation(out=pt, in_=sps, func=AF.Exp,
                                 bias=expb[:, 0:1], scale=SCALE)
            pts.append(pt)
            # PV sweep A (row tiles 0,1), one pair behind the exp
            if scp >= 1:
                for tsl in (0, 1):
                    nc.tensor.matmul(
                        apss[tsl],
                        lhsT=pts[scp - 1][:, :, ts(tsl, P)],
                        rhs=v_sb[:, 2 * (scp - 1):2 * scp, :],
                        start=(scp == 1), stop=False, perf_mode=DR,
                    )
            # interleave deferred transposes/output-proj of tb-1
            if scp >= 2:
                while todo and len(todo) > (7 - scp):
                    todo.pop(0)()
            # residual bias add for this block's x tiles (needed by the
            # output projection one block later)
            if scp >= 4:
                tt = tb * 4 + scp - 4
                nc.vector.tensor_add(out=x_sb[:, tt, :],
                                     in0=x_sb[:, tt, :], in1=bc_sb)
        for tsl in (0, 1):
            nc.tensor.matmul(
                apss[tsl], lhsT=pts[7][:, :, ts(tsl, P)],
                rhs=v_sb[:, 14:16, :], start=False, stop=True, perf_mode=DR,
            )
        while todo:
            todo.pop(0)()
        norm_cast(apss, abfs, 0)
        norm_cast(apss, abfs, 1)
        # PV sweep B (row tiles 2,3) over the retained p tiles
        for tsl in (2, 3):
            apss[tsl] = apsum.tile([P, VF], F32, tag="acc", name="apsB")
        for scp in range(8):
            for tsl in (2, 3):
                nc.tensor.matmul(
                    apss[tsl],
                    lhsT=pts[scp][:, :, ts(tsl, P)],
                    rhs=v_sb[:, 2 * scp:2 * scp + 2, :],
                    start=(scp == 0), stop=(scp == 7), perf_mode=DR,
                )
        norm_cast(apss, abfs, 2)
        norm_cast(apss, abfs, 3)
        # produce qT for the NEXT block inside this streak
        if tb + 1 < NTB:
            for uc in range(UCH):
                wps = apsum.tile([P, TBLK], F32, tag="misc", name="qps")
                for i in range(2):
                    nc.tensor.matmul(
                        wps,
                        lhsT=Wq_f8[:, 2 * i:2 * i + 2, ts(uc, P)],
                        rhs=xT_f8[:, 2 * i:2 * i + 2,
                                  ds((tb + 1) * TBLK, TBLK)],
                        start=(i == 0), stop=(i == 1), perf_mode=DR,
                    )
                nc.scalar.activation(
                    out=qT_f8[:, uc, ds((tb + 1) * TBLK, TBLK)],
                    in_=wps,
                    func=AF.Identity, bias=bq_sb[:, uc:uc + 1], scale=1.0,
                )
        if tb < NTB - 1:
            deferred = deferred_work(tb, abfs)
        else:
            # last block: emit immediately to shorten the tail
            for chunk in deferred_work(tb, abfs):
                chunk()

    for pool in (y_pool, rcp_pool, abf_pool, p_pool,
                 apsum, spsum, persist, consts):
        pool.release()


def _get_nc():
    if "nc" not in _cache:
        nc = bacc.Bacc("TRN2", target_bir_lowering=False, debug=False)
        with tile.TileContext(nc) as tc:
            _build_kernel(tc)
        nc.compile()
        _cache["nc"] = nc
    return _cache["nc"]


def _w8(w, chunks):
    """fp32 [K, N] -> fp8e4m3 [P, K//P, N] with K-chunk layout for lhsT."""
    f8 = w.reshape(chunks, P, -1).transpose(1, 0, 2)
    return np.ascontiguousarray(f8.astype(ml_dtypes.float8_e4m3))


def _host_inputs(inputs):
    f32 = np.float32
    Wa = np.ascontiguousarray(np.asarray(inputs["Wa"], dtype=f32))
    bc = np.asarray(inputs["bv"], dtype=f32) @ Wa + np.asarray(
        inputs["ba"], dtype=f32
    )
    bcrep = np.ascontiguousarray(
        np.broadcast_to(bc[None, :], (P, C)), dtype=f32
    )
    shared = {
        "Wq8": _w8(np.asarray(inputs["Wq"], dtype=f32), CCH),
        "Wk8": _w8(np.asarray(inputs["Wk"], dtype=f32), CCH),
        "Wv8": _w8(np.asarray(inputs["Wv"], dtype=f32), CCH),
        "Wa8": _w8(Wa, UCH),
        "bq": np.ascontiguousarray(np.asarray(inputs["bq"], dtype=f32)),
        "bk": np.ascontiguousarray(np.asarray(inputs["bk"], dtype=f32)),
        "bcrep": bcrep,
    }
    xs = np.ascontiguousarray(np.asarray(inputs["x"], dtype=f32))
    return [dict(shared, x=xs[b]) for b in range(B)]


def kernel(**inputs):
    nc = _get_nc()
    in_maps = _host_inputs(inputs)
    res = run_bass_kernel_spmd(nc, in_maps, core_ids=list(range(B)))
    return np.stack([res.results[b]["out"] for b in range(B)], axis=0)
